# revision 44
# baseline (speedup 1.0000x reference)
"""Fused multi-head attention (LN + QKV + softmax + out-proj) for TRN2,
sharded over 8 NeuronCores: batch (4) x head-group (2 groups of 6 heads).

Per core, for its (batch, head-group) shard:
    xn = LayerNorm(x[b])      (gamma+1 and 1/sqrt(dh) folded into weights)
    Q^T,K^T = W @ xn^T        (head-pair-packed, bf16 matmuls, f32 PSUM)
    V = xn @ Wv               (augmented with a ones column)
    per head, per query-half, per 128-key tile:
        S^T = K^T.T Q^T ; P^T = exp(S^T - C)   (ACT, fp8 out)
        attn^T[65,q] += [V|1].T P^T            (fp8 DoubleRow over key-tile pairs;
                                                row 64 = softmax denominators)
    normalize by row 64, out partial = attn^T.T @ WoT   (bf16 partial out)
Host sums the two partials per batch.
"""
import numpy as np

import concourse.bass as bass
import concourse.bacc as bacc
import concourse.tile as tile
from concourse import mybir
from concourse.bass_utils import run_bass_kernel_spmd

F32 = mybir.dt.float32
BF16 = mybir.dt.bfloat16
FP8 = mybir.dt.float8e4
AF = mybir.ActivationFunctionType
ALU = mybir.AluOpType

# ---- custom DVE exp2 (offloads part of the softmax exp from ScalarE) ----
# Scores arrive pre-scaled by log2(e) (folded into Wq on the host), so
# exp(s) = 2^y. Two DVE instructions at 1 elem/cycle each:
#   EXP2_BITS: k = round(y) via the +1.5*2^23 trick; writes (k+127)*2^23
#              to an int32 tile -> its bit pattern is the float 2^k.
#   EXP2_FRAC: f = y - round(y) in [-0.5, 0.5]; out = 2^k * (1 + f*(a + f*b))
# max rel err ~2e-3 (minimax quadratic for 2^f with the constant term fixed).
_RBIAS = 12582912.0          # 1.5 * 2^23
_PA, _PB = 0.70294179, 0.23986403
LN2 = 0.6931471805599453
LOG2E = 1.4426950408889634

_EXP_OPS = {}


def _register_exp_ops():
    if _EXP_OPS:
        return _EXP_OPS
    from concourse import dve_ops
    from concourse.dve_spec import Spec, Src0, Src1, C0, C1, C2, One, lower
    from concourse.dve_spec import _has_src1
    from concourse.dve_uop import DveOpSpec

    def _ref_bits(in0, in1, c0, c1, c2):
        y = in0.astype(np.float32)
        t = (y + np.float32(c0)).astype(np.float32)
        k = (t - np.float32(c0)).astype(np.float32)
        return (k * np.float32(c1) + np.float32(c2)).astype(np.float32)

    def _ref_frac(in0, in1, c0, c1, c2):
        y = in0.astype(np.float32)
        t = (y + np.float32(c0)).astype(np.float32)
        k = (t - np.float32(c0)).astype(np.float32)
        f = (y - k).astype(np.float32)
        return in1 * (1 + f * (np.float32(c1) + f * np.float32(c2)))

    t = Src0 + C0
    bits_body = (t - C0) * C1 + C2
    t2 = Src0 + C0
    f = Src0 - (t2 - C0)
    frac_body = Src1 * (One + f * (C1 + f * C2))

    ops = []
    for name, body, ref in (("EXP2_BITS_ATT", bits_body, _ref_bits),
                            ("EXP2_FRAC_ATT", frac_body, _ref_frac)):
        op = dve_ops.DveOp(name, Spec(body=body, reference=ref),
                           subdim=False, uops_sha={})
        dve_ops.OPS.append(op)
        dve_ops.CUSTOM_DVE_SPECS[name] = op.spec
        opcode = dve_ops._CUSTOM_DVE_ROW_BASE + len(dve_ops.OPS) - 1
        dve_ops._SUB_OPCODE_FOR_NAME[name] = opcode
        for ver in ("v3", "v4"):
            uops = lower(op.spec, ver=ver)
            op.uops_sha[ver] = DveOpSpec(
                name=name, opcode=opcode, uops=uops,
                rd1_en=_has_src1(op.spec)).sha(ver)
        ops.append(op)
    _EXP_OPS["bits"], _EXP_OPS["frac"] = ops
    return _EXP_OPS

B, N, DIM, H, DH = 4, 2048, 768, 12, 64
NCORES = 8
NH = 6            # heads per core
NP = 3            # head pairs per core
HCOLS = NH * DH   # 384
EXP_SHIFT = 3.0   # exp(s - C): keeps fp8 P in range; cancels in softmax

USE_FP8 = False
OUT_BF16 = True
DVE_EXP_KT = 0   # 0 = softmax exp entirely on ScalarE; k = offload kt%k==2 tiles to DVE


def build_graph(n=N, dim=DIM, num_devices=NCORES, use_fp8=USE_FP8,
                out_bf16=OUT_BF16):
    nt = n // 128        # token/key tiles
    ncdm = dim // 128    # dmodel chunks
    qhw = min(n, 1024)   # query-half width
    nqh = n // qhw

    nc = bacc.Bacc("TRN2", target_bir_lowering=False, debug=False,
                   num_devices=num_devices)
    x = nc.dram_tensor("x", [n, dim], F32, kind="ExternalInput").ap()
    wqt = nc.dram_tensor("wqt", [dim, HCOLS], BF16, kind="ExternalInput").ap()
    wkt = nc.dram_tensor("wkt", [dim, HCOLS], BF16, kind="ExternalInput").ap()
    wvt = nc.dram_tensor("wvt", [dim, HCOLS], BF16, kind="ExternalInput").ap()
    wot = nc.dram_tensor("wot", [HCOLS, dim], BF16, kind="ExternalInput").ap()
    ident = nc.dram_tensor("ident", [128, 128], BF16, kind="ExternalInput").ap()
    odt = BF16 if out_bf16 else F32
    out = nc.dram_tensor("out", [n, dim], odt, kind="ExternalOutput").ap()

    with tile.TileContext(nc) as tc:
        _body(tc, x, wqt, wkt, wvt, wot, ident, out,
              n, dim, nt, ncdm, qhw, nqh, use_fp8)
    nc.compile()
    return nc


def _body(tc, x, wqt, wkt, wvt, wot, ident, out,
          n, dim, nt, ncdm, qhw, nqh, use_fp8):
    nc = tc.nc
    from contextlib import ExitStack
    pdt = FP8 if use_fp8 else BF16
    with ExitStack() as ctx:
        consts = ctx.enter_context(tc.tile_pool(name="consts", bufs=1))
        sb = ctx.enter_context(tc.tile_pool(name="sb", bufs=1))
        xpool = ctx.enter_context(tc.tile_pool(name="xp", bufs=4))
        small = ctx.enter_context(tc.tile_pool(name="small", bufs=4))
        ppool = ctx.enter_context(tc.tile_pool(name="pp", bufs=4))
        rbpool = ctx.enter_context(tc.tile_pool(name="rb", bufs=3))
        oddp = ctx.enter_context(tc.tile_pool(name="odd", bufs=2))
        otp = ctx.enter_context(tc.tile_pool(name="ot", bufs=4))
        bitp = ctx.enter_context(tc.tile_pool(name="bitp", bufs=2))

        # constants + weights
        eps_sb = consts.tile([128, 1], F32, tag="eps")
        nc.vector.memset(eps_sb, 1e-5)
        id_sb = consts.tile([128, 128], BF16, tag="id")
        nc.sync.dma_start(out=id_sb, in_=ident)
        wq_sb = consts.tile([128, ncdm, HCOLS], BF16, tag="wq")
        nc.sync.dma_start(out=wq_sb, in_=wqt.rearrange("(c p) m -> p c m", p=128))
        wk_sb = consts.tile([128, ncdm, HCOLS], BF16, tag="wk")
        nc.sync.dma_start(out=wk_sb, in_=wkt.rearrange("(c p) m -> p c m", p=128))
        wv_sb = consts.tile([128, ncdm, HCOLS], BF16, tag="wv")
        nc.sync.dma_start(out=wv_sb, in_=wvt.rearrange("(c p) m -> p c m", p=128))
        wo_sb = consts.tile([128, NP, dim], BF16, tag="wo")
        nc.sync.dma_start(out=wo_sb, in_=wot.rearrange("(c p) m -> p c m", p=128))

        # persistent activations. K^T (the scores stationary) is stored per
        # head in full-128-row tiles: even heads occupy partitions 0-63
        # (64-127 zeroed), odd heads 64-127 (0-63 zeroed), so score matmuls
        # contract over K=128 with no PE row-tiling mode switches. Q^T (the
        # moving side) stays pair-packed: its other-half rows meet zeros in
        # the stationary, so their contents don't matter.
        xnT = sb.tile([128, ncdm, n], BF16, tag="xnT")
        qt_sb = sb.tile([128, NP, n], BF16, tag="qt")
        kt_sb = sb.tile([128, NH, n], BF16, tag="kt")
        for hh in range(NH):
            z0 = 64 if hh % 2 == 0 else 0
            nc.gpsimd.memset(kt_sb[z0:z0 + 64, hh, :], 0.0)
        if use_fp8:
            v_sb = sb.tile([128, NH, nt // 2, 2, DH + 1], pdt, tag="v")
            nc.vector.memset(v_sb[:, :, :, :, DH:DH + 1], 1.0)
        else:
            v_sb = sb.tile([128, NH, nt, DH + 1], pdt, tag="v")
            nc.vector.memset(v_sb[:, :, :, DH:DH + 1], 1.0)
        att_sb = sb.tile([128, NP, n], BF16, tag="att")

        x3 = x.rearrange("(t p) d -> t p d", p=128)
        out3 = out.rearrange("(t p) d -> t p d", p=128)

        # ---- phase 1: LayerNorm + transpose + Q/K/V projections ----
        # All x tiles are prefetched upfront; V projections and per-512-token
        # Q/K projections are interleaved into the LN loop so the PE stays
        # busy/warm; PSUM evictions are merged into few wide copies.
        # one tile per token block so LayerNorm(tt) only waits on its own DMA
        xtiles = []
        for tt in range(nt):
            xt_ = sb.tile([128, dim], F32, tag=f"xf{tt}")
            nc.sync.dma_start(out=xt_, in_=x3[tt])
            xtiles.append(xt_)
        with tc.tile_pool(name="psA", bufs=8, space="PSUM") as psA:
            for tt in range(nt):
                xt = xtiles[tt]
                ngr = dim // 256
                stats = small.tile([128, ngr, 6], F32, tag="stats")
                for g in range(ngr):
                    nc.vector.bn_stats(out=stats[:, g, :],
                                       in_=xt[:, g * 256:(g + 1) * 256])
                mv = small.tile([128, 2], F32, tag="mv")
                nc.vector.bn_aggr(out=mv, in_=stats)
                sq = small.tile([128, 1], F32, tag="sq")
                nc.scalar.activation(out=sq, in_=mv[:, 1:2], func=AF.Sqrt,
                                     bias=eps_sb)
                rstd = small.tile([128, 1], F32, tag="rstd")
                nc.vector.reciprocal(out=rstd, in_=sq)
                xn = xpool.tile([128, dim], BF16, tag="xn")
                nc.vector.tensor_scalar(out=xn, in0=xt, scalar1=mv[:, 0:1],
                                        scalar2=rstd, op0=ALU.subtract,
                                        op1=ALU.mult)
                ptt = psA.tile([128, ncdm, 128], BF16, tag="psA")
                for c in range(ncdm):
                    nc.tensor.transpose(ptt[:, c, :],
                                        xn[:, c * 128:(c + 1) * 128], id_sb)
                nc.vector.tensor_copy(out=xnT[:, :, tt * 128:(tt + 1) * 128],
                                      in_=ptt)
                for i in range(NP):
                    pst = psA.tile([128, 128], F32, tag="psA")
                    for c in range(ncdm):
                        nc.tensor.matmul(pst, xnT[:, c, tt * 128:(tt + 1) * 128],
                                         wv_sb[:, c, i * 128:(i + 1) * 128],
                                         start=(c == 0), stop=(c == ncdm - 1))
                    nc.vector.tensor_copy(
                        out=v_sb[:, 2 * i:2 * i + 2, tt, 0:DH],
                        in_=pst.rearrange("p (s d) -> p s d", d=DH))
                # Q/K projections for each completed 512-token column chunk
                if tt % 4 == 3:
                    cc = tt // 4
                    csl = slice(cc * 512, (cc + 1) * 512)
                    for i in range(NP):
                        pst = psA.tile([128, 512], F32, tag="psA")
                        for c in range(ncdm):
                            nc.tensor.matmul(pst,
                                             wq_sb[:, c, i * 128:(i + 1) * 128],
                                             xnT[:, c, csl],
                                             start=(c == 0), stop=(c == ncdm - 1))
                        nc.scalar.copy(out=qt_sb[:, i, csl], in_=pst)
                        pst = psA.tile([128, 512], F32, tag="psA")
                        for c in range(ncdm):
                            nc.tensor.matmul(pst,
                                             wk_sb[:, c, i * 128:(i + 1) * 128],
                                             xnT[:, c, csl],
                                             start=(c == 0), stop=(c == ncdm - 1))
                        nc.vector.tensor_copy(out=kt_sb[0:64, 2 * i, csl],
                                              in_=pst[0:64, :])
                        nc.scalar.copy(out=kt_sb[64:128, 2 * i + 1, csl],
                                       in_=pst[64:128, :])

        # ---- phase 2: attention ----
        with tc.tile_pool(name="psS", bufs=2, space="PSUM") as psS, \
             tc.tile_pool(name="psV", bufs=2, space="PSUM") as psV:
            for h in range(NH):
                i, s = h // 2, h % 2
                for qh in range(nqh):
                    q0 = qh * qhw
                    pv = psV.tile([65, qhw], F32, tag="pv")
                    for kt in range(nt):
                        p_t = ppool.tile([128, qhw], pdt, tag="p")
                        sc = psS.tile([128, qhw], F32, tag="sc")
                        for qq in range(qhw // 512):
                            nc.tensor.matmul(
                                sc[:, qq * 512:(qq + 1) * 512],
                                kt_sb[:, h, kt * 128:(kt + 1) * 128],
                                qt_sb[:, i, q0 + qq * 512:q0 + (qq + 1) * 512])
                        if DVE_EXP_KT and kt % DVE_EXP_KT == 2:
                            eo = _register_exp_ops()
                            bt = bitp.tile([128, qhw], mybir.dt.int32, tag="bits")
                            nc.vector._custom_dve(eo["bits"], out=bt, in0=sc,
                                                  s0=_RBIAS, s1=8388608.0,
                                                  imm2=1065353216.0)
                            nc.vector._custom_dve(eo["frac"], out=p_t, in0=sc,
                                                  in1=bt[:].bitcast(F32),
                                                  s0=_RBIAS, s1=_PA, imm2=_PB)
                        else:
                            nc.scalar.activation(out=p_t, in_=sc, func=AF.Exp,
                                                 scale=LN2)
                        for qq in range(qhw // 512):
                            nc.tensor.matmul(
                                pv[:, qq * 512:(qq + 1) * 512],
                                v_sb[:, h, kt, :],
                                p_t[:, qq * 512:(qq + 1) * 512],
                                start=(kt == 0), stop=(kt == nt - 1))
                    # normalize by softmax denominators (row 64)
                    srow = rbpool.tile([1, qhw], F32, tag="srow")
                    nc.vector.tensor_copy(out=srow, in_=pv[64:65, :])
                    rrow = rbpool.tile([1, qhw], F32, tag="srow")
                    nc.vector.reciprocal_approx_fast(out=rrow, in_=srow)
                    rc = rbpool.tile([64, qhw], F32, tag="rb")
                    nc.gpsimd.partition_broadcast(rc, rrow)
                    if s == 0:
                        nc.vector.tensor_mul(out=att_sb[0:64, i, q0:q0 + qhw],
                                             in0=pv[0:64, :], in1=rc)
                    else:
                        tmp = oddp.tile([64, qhw], BF16, tag="odd")
                        nc.vector.tensor_mul(out=tmp, in0=pv[0:64, :], in1=rc)
                        nc.sync.dma_start(out=att_sb[64:128, i, q0:q0 + qhw],
                                          in_=tmp)

        # ---- phase 3: output projection ----
        with tc.tile_pool(name="psO", bufs=3, space="PSUM") as psO:
            for tt in range(nt):
                po_t = psO.tile([128, dim], F32, tag="psO")
                for c in range(NP):
                    lhsT = att_sb[:, c, tt * 128:(tt + 1) * 128]
                    for o0 in range(0, dim, 512):
                        o1 = min(o0 + 512, dim)
                        nc.tensor.matmul(po_t[:, o0:o1], lhsT,
                                         wo_sb[:, c, o0:o1],
                                         start=(c == 0), stop=(c == NP - 1))
                ot = otp.tile([128, dim], out.dtype, tag="ot")
                if tt % 2 == 0:
                    nc.vector.tensor_copy(out=ot, in_=po_t)
                else:
                    nc.scalar.copy(out=ot, in_=po_t)
                nc.sync.dma_start(out=out3[tt], in_=ot)


_NC_CACHE = {}


def _get_nc():
    if "nc" not in _NC_CACHE:
        _NC_CACHE["nc"] = build_graph()
    return _NC_CACHE["nc"]


def make_in_maps(x, gamma, Wq, Wk, Wv, Wo):
    """Host-side sharding: core c -> batch c//2, head-group c%2."""
    import ml_dtypes
    bf16 = ml_dtypes.bfloat16
    g = (np.asarray(gamma, np.float32) + 1.0)
    scale = DH ** -0.5 * LOG2E  # scores computed in log2 domain
    Wq_eff = np.asarray(Wq, np.float32) * g[None, :] * scale
    Wk_eff = np.asarray(Wk, np.float32) * g[None, :]
    Wv_eff = np.asarray(Wv, np.float32)
    Wo_eff = np.asarray(Wo, np.float32)
    ident = np.eye(128, dtype=bf16)
    hg_maps = []
    for hg in range(2):
        r0, r1 = hg * HCOLS, (hg + 1) * HCOLS
        hg_maps.append({
            "wqt": np.ascontiguousarray(Wq_eff[r0:r1, :].T).astype(bf16),
            "wkt": np.ascontiguousarray(Wk_eff[r0:r1, :].T).astype(bf16),
            "wvt": np.ascontiguousarray(Wv_eff[r0:r1, :].T).astype(bf16),
            "wot": np.ascontiguousarray(Wo_eff[:, r0:r1].T).astype(bf16),
            "ident": ident,
        })
    in_maps = []
    for c in range(NCORES):
        b, hg = c // 2, c % 2
        m = dict(hg_maps[hg])
        m["x"] = np.ascontiguousarray(np.asarray(x, np.float32)[b])
        in_maps.append(m)
    return in_maps


def _run(inputs, trace=False, trace_kwargs=None):
    nc = _get_nc()
    in_maps = make_in_maps(**inputs)
    res = run_bass_kernel_spmd(nc, in_maps, core_ids=list(range(NCORES)),
                               trace=trace, **(trace_kwargs or {}))
    out = np.empty((B, N, DIM), np.float32)
    for b in range(B):
        out[b] = (res.results[2 * b]["out"].astype(np.float32)
                  + res.results[2 * b + 1]["out"].astype(np.float32))
    return out, res


def kernel(x, gamma, Wq, Wk, Wv, Wo):
    out, _ = _run(dict(x=x, gamma=gamma, Wq=Wq, Wk=Wk, Wv=Wv, Wo=Wo))
    return out


# revision 45
# speedup vs baseline: 1.0083x; 1.0083x over previous
"""Fused multi-head attention (LN + QKV + softmax + out-proj) for TRN2,
sharded over 8 NeuronCores: batch (4) x head-group (2 groups of 6 heads).

Per core, for its (batch, head-group) shard (all matmuls bf16, f32 PSUM):
    xn = LayerNorm(x[b])      (gamma+1, 1/sqrt(dh) and log2e folded into W on host)
    xn^T via PE transposes; Q^T,K^T = W @ xn^T; V = xn @ Wv (plus a ones column)
    per head, per 1024-query half, per 128-key tile kt:
        S^T[kt] = K^T(kt).T @ Q^T      (K^T per-head zero-padded to K=128 rows)
        P^T = exp2(S^T)                (ScalarE, scale=ln2)
        attn^T[65, q] += [V|1](kt).T @ P^T   (row 64 = softmax denominators)
    normalize by row 64 (fast reciprocal + gpsimd partition-broadcast),
    out partial = attn^T.T @ WoT       (bf16 partial, DMA out)
Host sums the two partials per batch.

The attention inner loop is ScalarE(exp)-bound and runs it at ~97% occupancy;
score/PV matmuls, PSUM evictions, normalizations and the output projection
overlap underneath via ping-ponged 2-bank PSUM score tiles and double-buffered
[65, 1024] PV accumulators (8 PSUM banks exactly).
"""
import numpy as np

import concourse.bass as bass
import concourse.bacc as bacc
import concourse.tile as tile
from concourse import mybir
from concourse.bass_utils import run_bass_kernel_spmd

F32 = mybir.dt.float32
BF16 = mybir.dt.bfloat16
FP8 = mybir.dt.float8e4
AF = mybir.ActivationFunctionType
ALU = mybir.AluOpType

# ---- custom DVE exp2 (offloads part of the softmax exp from ScalarE) ----
# Scores arrive pre-scaled by log2(e) (folded into Wq on the host), so
# exp(s) = 2^y. Two DVE instructions at 1 elem/cycle each:
#   EXP2_BITS: k = round(y) via the +1.5*2^23 trick; writes (k+127)*2^23
#              to an int32 tile -> its bit pattern is the float 2^k.
#   EXP2_FRAC: f = y - round(y) in [-0.5, 0.5]; out = 2^k * (1 + f*(a + f*b))
# max rel err ~2e-3 (minimax quadratic for 2^f with the constant term fixed).
_RBIAS = 12582912.0          # 1.5 * 2^23
_PA, _PB = 0.70294179, 0.23986403
LN2 = 0.6931471805599453
LOG2E = 1.4426950408889634

_EXP_OPS = {}


def _register_exp_ops():
    if _EXP_OPS:
        return _EXP_OPS
    from concourse import dve_ops
    from concourse.dve_spec import Spec, Src0, Src1, C0, C1, C2, One, lower
    from concourse.dve_spec import _has_src1
    from concourse.dve_uop import DveOpSpec

    def _ref_bits(in0, in1, c0, c1, c2):
        y = in0.astype(np.float32)
        t = (y + np.float32(c0)).astype(np.float32)
        k = (t - np.float32(c0)).astype(np.float32)
        return (k * np.float32(c1) + np.float32(c2)).astype(np.float32)

    def _ref_frac(in0, in1, c0, c1, c2):
        y = in0.astype(np.float32)
        t = (y + np.float32(c0)).astype(np.float32)
        k = (t - np.float32(c0)).astype(np.float32)
        f = (y - k).astype(np.float32)
        return in1 * (1 + f * (np.float32(c1) + f * np.float32(c2)))

    t = Src0 + C0
    bits_body = (t - C0) * C1 + C2
    t2 = Src0 + C0
    f = Src0 - (t2 - C0)
    frac_body = Src1 * (One + f * (C1 + f * C2))

    ops = []
    for name, body, ref in (("EXP2_BITS_ATT", bits_body, _ref_bits),
                            ("EXP2_FRAC_ATT", frac_body, _ref_frac)):
        op = dve_ops.DveOp(name, Spec(body=body, reference=ref),
                           subdim=False, uops_sha={})
        dve_ops.OPS.append(op)
        dve_ops.CUSTOM_DVE_SPECS[name] = op.spec
        opcode = dve_ops._CUSTOM_DVE_ROW_BASE + len(dve_ops.OPS) - 1
        dve_ops._SUB_OPCODE_FOR_NAME[name] = opcode
        for ver in ("v3", "v4"):
            uops = lower(op.spec, ver=ver)
            op.uops_sha[ver] = DveOpSpec(
                name=name, opcode=opcode, uops=uops,
                rd1_en=_has_src1(op.spec)).sha(ver)
        ops.append(op)
    _EXP_OPS["bits"], _EXP_OPS["frac"] = ops
    return _EXP_OPS

B, N, DIM, H, DH = 4, 2048, 768, 12, 64
NCORES = 8
NH = 6            # heads per core
NP = 3            # head pairs per core
HCOLS = NH * DH   # 384
EXP_SHIFT = 3.0   # exp(s - C): keeps fp8 P in range; cancels in softmax

USE_FP8 = False
OUT_BF16 = True
DVE_EXP_KT = 0   # 0 = softmax exp entirely on ScalarE; k = offload kt%k==2 tiles to DVE


def build_graph(n=N, dim=DIM, num_devices=NCORES, use_fp8=USE_FP8,
                out_bf16=OUT_BF16):
    nt = n // 128        # token/key tiles
    ncdm = dim // 128    # dmodel chunks
    qhw = min(n, 1024)   # query-half width
    nqh = n // qhw

    nc = bacc.Bacc("TRN2", target_bir_lowering=False, debug=False,
                   num_devices=num_devices)
    x = nc.dram_tensor("x", [n, dim], F32, kind="ExternalInput").ap()
    wqt = nc.dram_tensor("wqt", [dim, HCOLS], BF16, kind="ExternalInput").ap()
    wkt = nc.dram_tensor("wkt", [dim, HCOLS], BF16, kind="ExternalInput").ap()
    wvt = nc.dram_tensor("wvt", [dim, HCOLS], BF16, kind="ExternalInput").ap()
    wot = nc.dram_tensor("wot", [HCOLS, dim], BF16, kind="ExternalInput").ap()
    ident = nc.dram_tensor("ident", [128, 128], BF16, kind="ExternalInput").ap()
    odt = BF16 if out_bf16 else F32
    out = nc.dram_tensor("out", [n, dim], odt, kind="ExternalOutput").ap()

    with tile.TileContext(nc) as tc:
        _body(tc, x, wqt, wkt, wvt, wot, ident, out,
              n, dim, nt, ncdm, qhw, nqh, use_fp8)
    nc.compile()
    return nc


def _body(tc, x, wqt, wkt, wvt, wot, ident, out,
          n, dim, nt, ncdm, qhw, nqh, use_fp8):
    nc = tc.nc
    from contextlib import ExitStack
    pdt = FP8 if use_fp8 else BF16
    with ExitStack() as ctx:
        consts = ctx.enter_context(tc.tile_pool(name="consts", bufs=1))
        sb = ctx.enter_context(tc.tile_pool(name="sb", bufs=1))
        xpool = ctx.enter_context(tc.tile_pool(name="xp", bufs=4))
        small = ctx.enter_context(tc.tile_pool(name="small", bufs=4))
        ppool = ctx.enter_context(tc.tile_pool(name="pp", bufs=4))
        rbpool = ctx.enter_context(tc.tile_pool(name="rb", bufs=3))
        oddp = ctx.enter_context(tc.tile_pool(name="odd", bufs=2))
        otp = ctx.enter_context(tc.tile_pool(name="ot", bufs=4))
        bitp = ctx.enter_context(tc.tile_pool(name="bitp", bufs=2))

        # constants + weights
        eps_sb = consts.tile([128, 1], F32, tag="eps")
        nc.vector.memset(eps_sb, 1e-5)
        id_sb = consts.tile([128, 128], BF16, tag="id")
        nc.sync.dma_start(out=id_sb, in_=ident)
        wq_sb = consts.tile([128, ncdm, HCOLS], BF16, tag="wq")
        nc.sync.dma_start(out=wq_sb, in_=wqt.rearrange("(c p) m -> p c m", p=128))
        wk_sb = consts.tile([128, ncdm, HCOLS], BF16, tag="wk")
        nc.sync.dma_start(out=wk_sb, in_=wkt.rearrange("(c p) m -> p c m", p=128))
        wv_sb = consts.tile([128, ncdm, HCOLS], BF16, tag="wv")
        nc.sync.dma_start(out=wv_sb, in_=wvt.rearrange("(c p) m -> p c m", p=128))
        wo_sb = consts.tile([128, NP, dim], BF16, tag="wo")
        nc.sync.dma_start(out=wo_sb, in_=wot.rearrange("(c p) m -> p c m", p=128))

        # persistent activations. K^T (the scores stationary) is stored per
        # head in full-128-row tiles: even heads occupy partitions 0-63
        # (64-127 zeroed), odd heads 64-127 (0-63 zeroed), so score matmuls
        # contract over K=128 with no PE row-tiling mode switches. Q^T (the
        # moving side) stays pair-packed: its other-half rows meet zeros in
        # the stationary, so their contents don't matter.
        xnT = sb.tile([128, ncdm, n], BF16, tag="xnT")
        qt_sb = sb.tile([128, NP, n], BF16, tag="qt")
        kt_sb = sb.tile([128, NH, n], BF16, tag="kt")
        for hh in range(NH):
            z0 = 64 if hh % 2 == 0 else 0
            nc.gpsimd.memset(kt_sb[z0:z0 + 64, hh, :], 0.0)
        if use_fp8:
            v_sb = sb.tile([128, NH, nt // 2, 2, DH + 1], pdt, tag="v")
            nc.vector.memset(v_sb[:, :, :, :, DH:DH + 1], 1.0)
        else:
            v_sb = sb.tile([128, NH, nt, DH + 1], pdt, tag="v")
            nc.vector.memset(v_sb[:, :, :, DH:DH + 1], 1.0)
        att_sb = sb.tile([128, NP, n], BF16, tag="att")

        x3 = x.rearrange("(t p) d -> t p d", p=128)
        out3 = out.rearrange("(t p) d -> t p d", p=128)

        # ---- phase 1: LayerNorm + transpose + Q/K/V projections ----
        # All x tiles are prefetched upfront; V projections and per-512-token
        # Q/K projections are interleaved into the LN loop so the PE stays
        # busy/warm; PSUM evictions are merged into few wide copies.
        # one tile per token block so LayerNorm(tt) only waits on its own DMA
        xtiles = []
        for tt in range(nt):
            xt_ = sb.tile([128, dim], F32, tag=f"xf{tt}")
            nc.sync.dma_start(out=xt_, in_=x3[tt])
            xtiles.append(xt_)
        with tc.tile_pool(name="psA", bufs=8, space="PSUM") as psA:
            for tt in range(nt):
                xt = xtiles[tt]
                ngr = dim // 256
                stats = small.tile([128, ngr, 6], F32, tag="stats")
                for g in range(ngr):
                    nc.vector.bn_stats(out=stats[:, g, :],
                                       in_=xt[:, g * 256:(g + 1) * 256])
                mv = small.tile([128, 2], F32, tag="mv")
                nc.vector.bn_aggr(out=mv, in_=stats)
                sq = small.tile([128, 1], F32, tag="sq")
                nc.scalar.activation(out=sq, in_=mv[:, 1:2], func=AF.Sqrt,
                                     bias=eps_sb)
                rstd = small.tile([128, 1], F32, tag="rstd")
                nc.vector.reciprocal(out=rstd, in_=sq)
                xn = xpool.tile([128, dim], BF16, tag="xn")
                nc.vector.tensor_scalar(out=xn, in0=xt, scalar1=mv[:, 0:1],
                                        scalar2=rstd, op0=ALU.subtract,
                                        op1=ALU.mult)
                ptt = psA.tile([128, ncdm, 128], BF16, tag="psA")
                for c in range(ncdm):
                    nc.tensor.transpose(ptt[:, c, :],
                                        xn[:, c * 128:(c + 1) * 128], id_sb)
                nc.vector.tensor_copy(out=xnT[:, :, tt * 128:(tt + 1) * 128],
                                      in_=ptt)
                for i in range(NP):
                    pst = psA.tile([128, 128], F32, tag="psA")
                    for c in range(ncdm):
                        nc.tensor.matmul(pst, xnT[:, c, tt * 128:(tt + 1) * 128],
                                         wv_sb[:, c, i * 128:(i + 1) * 128],
                                         start=(c == 0), stop=(c == ncdm - 1))
                    nc.vector.tensor_copy(
                        out=v_sb[:, 2 * i:2 * i + 2, tt, 0:DH],
                        in_=pst.rearrange("p (s d) -> p s d", d=DH))
                # Q/K projections for each completed 512-token column chunk
                if tt % 4 == 3:
                    cc = tt // 4
                    csl = slice(cc * 512, (cc + 1) * 512)
                    for i in range(NP):
                        pst = psA.tile([128, 512], F32, tag="psA")
                        for c in range(ncdm):
                            nc.tensor.matmul(pst,
                                             wq_sb[:, c, i * 128:(i + 1) * 128],
                                             xnT[:, c, csl],
                                             start=(c == 0), stop=(c == ncdm - 1))
                        nc.scalar.copy(out=qt_sb[:, i, csl], in_=pst)
                        pst = psA.tile([128, 512], F32, tag="psA")
                        for c in range(ncdm):
                            nc.tensor.matmul(pst,
                                             wk_sb[:, c, i * 128:(i + 1) * 128],
                                             xnT[:, c, csl],
                                             start=(c == 0), stop=(c == ncdm - 1))
                        nc.vector.tensor_copy(out=kt_sb[0:64, 2 * i, csl],
                                              in_=pst[0:64, :])
                        nc.scalar.copy(out=kt_sb[64:128, 2 * i + 1, csl],
                                       in_=pst[64:128, :])

        # ---- phase 2: attention ----
        with tc.tile_pool(name="psS", bufs=2, space="PSUM") as psS, \
             tc.tile_pool(name="psV", bufs=2, space="PSUM") as psV:
            for h in range(NH):
                i, s = h // 2, h % 2
                for qh in range(nqh):
                    q0 = qh * qhw
                    pv = psV.tile([65, qhw], F32, tag="pv")
                    for kt in range(nt):
                        p_t = ppool.tile([128, qhw], pdt, tag="p")
                        sc = psS.tile([128, qhw], F32, tag="sc")
                        for qq in range(qhw // 512):
                            nc.tensor.matmul(
                                sc[:, qq * 512:(qq + 1) * 512],
                                kt_sb[:, h, kt * 128:(kt + 1) * 128],
                                qt_sb[:, i, q0 + qq * 512:q0 + (qq + 1) * 512])
                        if DVE_EXP_KT and kt % DVE_EXP_KT == 2:
                            eo = _register_exp_ops()
                            bt = bitp.tile([128, qhw], mybir.dt.int32, tag="bits")
                            nc.vector._custom_dve(eo["bits"], out=bt, in0=sc,
                                                  s0=_RBIAS, s1=8388608.0,
                                                  imm2=1065353216.0)
                            nc.vector._custom_dve(eo["frac"], out=p_t, in0=sc,
                                                  in1=bt[:].bitcast(F32),
                                                  s0=_RBIAS, s1=_PA, imm2=_PB)
                        else:
                            nc.scalar.activation(out=p_t, in_=sc, func=AF.Exp,
                                                 scale=LN2)
                        for qq in range(qhw // 512):
                            nc.tensor.matmul(
                                pv[:, qq * 512:(qq + 1) * 512],
                                v_sb[:, h, kt, :],
                                p_t[:, qq * 512:(qq + 1) * 512],
                                start=(kt == 0), stop=(kt == nt - 1))
                    # normalize by softmax denominators (row 64)
                    srow = rbpool.tile([1, qhw], F32, tag="srow")
                    nc.vector.tensor_copy(out=srow, in_=pv[64:65, :])
                    rrow = rbpool.tile([1, qhw], F32, tag="srow")
                    nc.vector.reciprocal_approx_fast(out=rrow, in_=srow)
                    rc = rbpool.tile([64, qhw], F32, tag="rb")
                    nc.gpsimd.partition_broadcast(rc, rrow)
                    if s == 0:
                        nc.vector.tensor_mul(out=att_sb[0:64, i, q0:q0 + qhw],
                                             in0=pv[0:64, :], in1=rc)
                    else:
                        tmp = oddp.tile([64, qhw], BF16, tag="odd")
                        nc.vector.tensor_mul(out=tmp, in0=pv[0:64, :], in1=rc)
                        nc.sync.dma_start(out=att_sb[64:128, i, q0:q0 + qhw],
                                          in_=tmp)

        # ---- phase 3: output projection ----
        with tc.tile_pool(name="psO", bufs=3, space="PSUM") as psO:
            for tt in range(nt):
                po_t = psO.tile([128, dim], F32, tag="psO")
                for c in range(NP):
                    lhsT = att_sb[:, c, tt * 128:(tt + 1) * 128]
                    for o0 in range(0, dim, 512):
                        o1 = min(o0 + 512, dim)
                        nc.tensor.matmul(po_t[:, o0:o1], lhsT,
                                         wo_sb[:, c, o0:o1],
                                         start=(c == 0), stop=(c == NP - 1))
                ot = otp.tile([128, dim], out.dtype, tag="ot")
                if tt % 2 == 0:
                    nc.vector.tensor_copy(out=ot, in_=po_t)
                else:
                    nc.scalar.copy(out=ot, in_=po_t)
                nc.sync.dma_start(out=out3[tt], in_=ot)


_NC_CACHE = {}


def _get_nc():
    if "nc" not in _NC_CACHE:
        _NC_CACHE["nc"] = build_graph()
    return _NC_CACHE["nc"]


def make_in_maps(x, gamma, Wq, Wk, Wv, Wo):
    """Host-side sharding: core c -> batch c//2, head-group c%2."""
    import ml_dtypes
    bf16 = ml_dtypes.bfloat16
    g = (np.asarray(gamma, np.float32) + 1.0)
    scale = DH ** -0.5 * LOG2E  # scores computed in log2 domain
    Wq_eff = np.asarray(Wq, np.float32) * g[None, :] * scale
    Wk_eff = np.asarray(Wk, np.float32) * g[None, :]
    Wv_eff = np.asarray(Wv, np.float32)
    Wo_eff = np.asarray(Wo, np.float32)
    ident = np.eye(128, dtype=bf16)
    hg_maps = []
    for hg in range(2):
        r0, r1 = hg * HCOLS, (hg + 1) * HCOLS
        hg_maps.append({
            "wqt": np.ascontiguousarray(Wq_eff[r0:r1, :].T).astype(bf16),
            "wkt": np.ascontiguousarray(Wk_eff[r0:r1, :].T).astype(bf16),
            "wvt": np.ascontiguousarray(Wv_eff[r0:r1, :].T).astype(bf16),
            "wot": np.ascontiguousarray(Wo_eff[:, r0:r1].T).astype(bf16),
            "ident": ident,
        })
    in_maps = []
    for c in range(NCORES):
        b, hg = c // 2, c % 2
        m = dict(hg_maps[hg])
        m["x"] = np.ascontiguousarray(np.asarray(x, np.float32)[b])
        in_maps.append(m)
    return in_maps


def _run(inputs, trace=False, trace_kwargs=None):
    nc = _get_nc()
    in_maps = make_in_maps(**inputs)
    res = run_bass_kernel_spmd(nc, in_maps, core_ids=list(range(NCORES)),
                               trace=trace, **(trace_kwargs or {}))
    out = np.empty((B, N, DIM), np.float32)
    for b in range(B):
        out[b] = (res.results[2 * b]["out"].astype(np.float32)
                  + res.results[2 * b + 1]["out"].astype(np.float32))
    return out, res


def kernel(x, gamma, Wq, Wk, Wv, Wo):
    out, _ = _run(dict(x=x, gamma=gamma, Wq=Wq, Wk=Wk, Wv=Wv, Wo=Wo))
    return out
